# revision 25
# baseline (speedup 1.0000x reference)
"""AttentivePooling Trainium2 kernel (chunk-major streaming, bf16).

Reference semantics (h_all: [T, B, D] f32, xin unused):
    h_last = h_all[-1]                       # [B, D]
    a[b, t] = <h_all[t, b, :], h_last[b, :]> / sqrt(D)
    r = relu(a)
    w = r / (sum_t r + 1e-9)
    out[b, d] = sum_t w[b, t] * h_all[t, b, d]

The 1/sqrt(D) scale cancels in w (relu is positively homogeneous) and
the eps is negligible: a[T-1] = |h_last|^2/sqrt(D) > 0 always, so the
relu-sum is >> 1e-9.  We pool with unnormalized relu scores and divide
by their sum at the end.

Layout: data-parallel over B across 8 cores (8 batches/core).  Per
core the shard h[:, 0:8, :] is streamed t-chunk-major: chunk c is the
fully contiguous DRAM block h[128c:128c+128, :, :] -> one 2MB SWDGE
DMA (f32 read, inline cast to bf16) into an SBUF tile
[128(t), 4096(b*d)].  HBM traffic is unchanged (f32 reads); bf16
on-chip doubles DVE throughput and, critically, lets the pooling
matmuls target PSUM partition quadrants (f32r matmuls may only write
partition 0; bf16 may write 0/32/64 - walrus s3d3_mm_valid_dst_partition).
Per chunk:
  - scores: tmp = h_c * hlb (hlb = h_last broadcast to 128 partitions,
    built once at startup via PE ones-outer-products + PSUM->SBUF
    casts); multiply split DVE (b 0-5, two ops) / GPSIMD (b 6-7);
    d-reduce split ACT activation-accum (b 0-5, f32 accum) / DVE
    segmented tensor_reduce (b 6-7, f32 out)
  - relu -> w_c [128, 8] bf16 (unnormalized weights)
  - PE (bf16): per b accumulate w_c[:,b]^T @ h_c[:,b-block] into PSUM
    row (b%3)*32 of tile b//3 (3 [96,512] tiles hold 8 rows in 3
    banks); plus w_c^T @ ones -> zrow [8,1] f32 (relu-score sums)
Epilogue: scatter zrow to the same quadrant rows via an f32r 0/1
selector matmul (keeps the reciprocal and the per-partition scale
lane-aligned with pout), zrec = 1/z (DVE), res = pout * zrec (one ACT
per tile over all 96 partitions; junk rows unused), 3 partition-strided
2KB stores.  All normalization math stays f32.
"""

import numpy as np
from contextlib import ExitStack

import concourse.bass as bass
import concourse.tile as tile
from concourse import bacc, mybir
from concourse.bass_utils import run_bass_kernel_spmd

T, B, D = 2048, 64, 512
NCORES = 8
BPC = B // NCORES  # batches per core
P = 128
NCHUNK = T // P  # 16
FB = BPC * D  # 4096 free elems per partition per chunk
DVE_B = 5  # batches 0..4 multiplied on DVE
GP_B = BPC - DVE_B  # batches 5..7 multiplied on GPSIMD
ACT_R = 5  # batches 0..4 reduced on ACT; b 5..7 segmented-reduced on DVE
PREFETCH = 6  # chunks in flight

_nc_cache = None


def _build():
    global _nc_cache
    if _nc_cache is not None:
        return _nc_cache
    nc = bacc.Bacc("TRN2", debug=False, target_bir_lowering=False, num_devices=NCORES)
    h = nc.dram_tensor("h", [T, BPC, D], mybir.dt.float32, kind="ExternalInput")
    # selt[t][b, (b%3)*32] = 1 for b//3 == t: scatters zrow to the
    # quadrant rows used by the pooling matmuls (engine ops cannot
    # address partition bases other than 0/32/64/96, so this constant
    # comes from the host instead of per-element memsets)
    selt = nc.dram_tensor("selt", [P, 3 * 96], mybir.dt.float32, kind="ExternalInput")
    out = nc.dram_tensor("out", [BPC, D], mybir.dt.float32, kind="ExternalOutput")
    h_ap = h.ap()
    out_ap = out.ap()
    f32 = mybir.dt.float32
    f32r = mybir.dt.float32r
    bf16 = mybir.dt.bfloat16

    def quad(b):  # pooling row for batch b: PSUM tile b//3, partition (b%3)*32
        return b // 3, (b % 3) * 32

    with tile.TileContext(nc) as tc:
        with ExitStack() as ctx:
            hpool = ctx.enter_context(tc.tile_pool(name="h", bufs=PREFETCH + 1))
            tmpap = ctx.enter_context(tc.tile_pool(name="tmpa", bufs=2))
            tmpbp = ctx.enter_context(tc.tile_pool(name="tmpb", bufs=2))
            scrp = ctx.enter_context(tc.tile_pool(name="scr", bufs=3))
            constp = ctx.enter_context(tc.tile_pool(name="const", bufs=1))
            resp = ctx.enter_context(tc.tile_pool(name="res", bufs=3))
            psbcp = ctx.enter_context(tc.tile_pool(name="psbc", bufs=2, space="PSUM"))
            psoutp = ctx.enter_context(tc.tile_pool(name="pso", bufs=1, space="PSUM"))
            pszp = ctx.enter_context(tc.tile_pool(name="psz", bufs=1, space="PSUM"))
            zpsp = ctx.enter_context(tc.tile_pool(name="zps", bufs=1, space="PSUM"))

            ones_f = constp.tile([P, 1], f32)
            nc.vector.memset(ones_f[:], 1.0)
            ones_row_f = constp.tile([1, P], f32)
            nc.vector.memset(ones_row_f[:], 1.0)
            ones_row = constp.tile([1, P], bf16)
            nc.vector.tensor_copy(ones_row[:], ones_row_f[:])
            ones_col = constp.tile([P, 1], bf16)
            nc.vector.tensor_copy(ones_col[:], ones_f[:])

            # selector matrices (from host): sel[t] = sel_sb[:, 96t:96t+96]
            # (plain f32 matmul: f32r requires rhs free size >= 2 and the
            # scatter streams a single column; rows >= BPC are zero)
            sel_sb = constp.tile([P, 3 * 96], f32)
            nc.sync.dma_start(sel_sb[:], selt.ap())

            # h_last = h[T-1, :, :] -> one partition (cast to bf16), then
            # PE-broadcast to all 128 partitions, PSUM -> SBUF bf16.
            hl1 = constp.tile([1, FB], bf16)
            nc.gpsimd.dma_start(
                hl1[:], h_ap[T - 1 : T, :, :].rearrange("t b d -> t (b d)")
            )
            hlb = constp.tile([P, FB], bf16)
            for b in range(BPC):
                pbc = psbcp.tile([P, D], f32, tag="pbc")
                nc.tensor.matmul(
                    pbc[:],
                    ones_row[:],
                    hl1[0:1, b * D : (b + 1) * D],
                    start=True,
                    stop=True,
                )
                if b % 2 == 0:
                    nc.scalar.copy(hlb[:, b * D : (b + 1) * D], pbc[:])
                else:
                    nc.vector.tensor_copy(hlb[:, b * D : (b + 1) * D], pbc[:])

            pouts = [
                psoutp.tile([96, D], f32, tag=f"pout{t}", name=f"pout{t}")
                for t in range(3)
            ]
            zrow = pszp.tile([BPC, 1], f32)

            def load(c):
                t_ = hpool.tile([P, FB], bf16, tag="hsb", name="h_sb")
                nc.gpsimd.dma_start(
                    t_[:],
                    h_ap[c * P : (c + 1) * P, :, :].rearrange("t b d -> t (b d)"),
                )
                return t_

            h_tiles = {}
            for c in range(min(PREFETCH, NCHUNK)):
                h_tiles[c] = load(c)

            for c in range(NCHUNK):
                h_sb = h_tiles.pop(c)
                first = c == 0
                last = c == NCHUNK - 1

                # elementwise h * h_last_broadcast (bf16, DVE 2x mode)
                tmpa = tmpap.tile([P, DVE_B * D], bf16, tag="tmpa")
                tmpb = tmpbp.tile([P, GP_B * D], bf16, tag="tmpb")
                nc.gpsimd.tensor_tensor(
                    tmpb[:],
                    h_sb[:, DVE_B * D : FB],
                    hlb[:, DVE_B * D : FB],
                    mybir.AluOpType.mult,
                )
                # DVE's half in two ops so ACT reductions start earlier
                H2 = DVE_B // 2
                nc.vector.tensor_tensor(
                    tmpa[:, 0 : H2 * D],
                    h_sb[:, 0 : H2 * D],
                    hlb[:, 0 : H2 * D],
                    mybir.AluOpType.mult,
                )
                nc.vector.tensor_tensor(
                    tmpa[:, H2 * D : DVE_B * D],
                    h_sb[:, H2 * D : DVE_B * D],
                    hlb[:, H2 * D : DVE_B * D],
                    mybir.AluOpType.mult,
                )

                # reduce over d: scr[p, b] = sum_d tmp[p, b*D+d].  ACT
                # accum costs ~810ns/block (no bf16 speedup) so it only
                # gets 3 blocks; DVE segmented reduces take the rest.
                scr = scrp.tile([P, BPC], f32, tag="scr")
                for b in range(ACT_R):
                    nc.scalar.activation(
                        tmpa[:, b * D : (b + 1) * D],
                        tmpa[:, b * D : (b + 1) * D],
                        mybir.ActivationFunctionType.Copy,
                        accum_out=scr[:, b : b + 1],
                    )
                nc.vector.tensor_reduce(
                    scr[:, DVE_B:BPC],
                    tmpb[:].rearrange("p (b d) -> p b d", b=GP_B),
                    mybir.AxisListType.X,
                    mybir.AluOpType.add,
                )

                # relu -> unnormalized weights (bf16)
                w_c = scrp.tile([P, BPC], bf16, tag="w")
                nc.scalar.activation(w_c[:], scr[:], mybir.ActivationFunctionType.Relu)

                # pooling: pout row of b += w_c[:,b]^T @ h_c[:,b-block]
                for b in range(BPC):
                    t, q = quad(b)
                    nc.tensor.matmul(
                        pouts[t][q : q + 1, :],
                        w_c[:, b : b + 1],
                        h_sb[:, b * D : (b + 1) * D],
                        start=first,
                        stop=last,
                    )
                nc.tensor.matmul(zrow[:], w_c[:], ones_col[:], start=first, stop=last)

                if c + PREFETCH < NCHUNK:
                    h_tiles[c + PREFETCH] = load(c + PREFETCH)

            # epilogue: out = pout / z  (eps negligible, see header)
            zrow_f = scrp.tile([P, 1], f32, tag="zf")
            nc.vector.memset(zrow_f[:], 0.0)
            nc.vector.tensor_copy(zrow_f[0:BPC, :], zrow[:])
            for t in range(3):
                zps = zpsp.tile([96, 1], f32, tag="zps")
                nc.tensor.matmul(
                    zps[:],
                    sel_sb[:, 96 * t : 96 * (t + 1)],
                    zrow_f[:],
                    start=True,
                    stop=True,
                )
                zrec = scrp.tile([96, 1], f32, tag=f"zrec{t}")
                nc.vector.reciprocal(zrec[:], zps[:])
                res = resp.tile([96, D], f32, tag=f"res{t}", name="res")
                nc.scalar.activation(
                    res[:],
                    pouts[t][:],
                    mybir.ActivationFunctionType.Copy,
                    scale=zrec[:, 0:1],
                )
                nb = min(BPC - 3 * t, 3)  # rows used in this tile (3, 3, 2)
                src = res[:].rearrange("(g r) d -> g r d", r=32)[0:nb, 0, :]
                nc.sync.dma_start(out_ap[3 * t : 3 * t + nb, :], src)

    nc.finalize()
    _nc_cache = nc
    return nc


def _run(h_all: np.ndarray, trace: bool = False):
    nc = _build()
    h_all = np.ascontiguousarray(np.asarray(h_all), dtype=np.float32)
    assert h_all.shape == (T, B, D)
    sel_np = np.zeros((P, 3 * 96), dtype=np.float32)
    for b in range(BPC):
        sel_np[b, 96 * (b // 3) + (b % 3) * 32] = 1.0
    in_maps = [
        {
            "h": np.ascontiguousarray(h_all[:, c * BPC : (c + 1) * BPC, :]),
            "selt": sel_np,
        }
        for c in range(NCORES)
    ]
    r = run_bass_kernel_spmd(nc, in_maps, list(range(NCORES)), trace=trace)
    out = np.concatenate([r.results[c]["out"] for c in range(NCORES)], axis=0)
    return out, r


def kernel(h_all: np.ndarray, xin: np.ndarray | None = None) -> np.ndarray:
    out, _ = _run(h_all)
    return out
